# revision 1
# baseline (speedup 1.0000x reference)
"""Trainium2 Bass kernel for DifferentiableSupergraphDynamics.

Computation:
    edge_w = where(learn_mask, tanh(theta), sign*conf) * delay      [E]
    msgs   = x[:, src] * edge_w                                     [B, E]
    agg    = scatter_add(msgs -> dst)                               [B, N]
    rate   = base_rate * exp(rate_log_scale)                        [N]
    drive  = tanh(agg + bias)
    x_next = clip(x + DT * rate * (drive*cap - x), 0, cap)

Sharding: destination nodes are dealt round-robin (by total in-degree
rank) across the 8 cores; every edge lives on its destination's core, so
no cross-core collective is needed.

Per-core edge phase: edges are split into (up to) 4 "structures" by
source-node range (32768 rows each, so dma_gather's int16 indices can
address the x table). Each structure is a padded CSR over the core's
nodes sorted by that structure's in-degree: node groups of 128
partitions padded to the group max degree D. The x-row gather for all of
a structure's slots is done with the vectorized SWDGE dma_gather ucode
(one 64B descriptor per slot, round-robin over the 4 SWDGE queues), the
weighted per-node reduction is a strided Vector-engine tensor_reduce,
and the 4 per-structure partial aggregates are merged into structure-0's
node order with unique-index dma_scatter_add through HBM.
"""

import numpy as np

import concourse.bass as bass
import concourse.bacc as bacc
import concourse.mybir as mybir
import concourse.tile as tile
from concourse.bass_utils import run_bass_kernel_spmd

P = 128
NCORES = 8
DT = 0.1
SRC_CHUNK = 32768          # dma_gather int16 index reach
GATHER_CALL = 8192         # SWDGE ring capacity per call
XBF16 = False              # f32 x rows (64B descriptors); bf16 trips clip-boundary rel err
ROWE = 64                  # x-table row stride: 256B (dma_gather req)
XROW = 128 if XBF16 else 64

F32 = mybir.dt.float32
I16 = mybir.dt.int16
I8 = mybir.dt.int8


def _wrap_idx(flat, call):
    """Lay a flat int16 index list out in the SWDGE wrapped layout:
    per call of `call` indices, index j at [j%16, j//16]; 32-partition
    replicated (descriptor-gen runs on two Q7 cores)."""
    n = len(flat)
    ncall = (n + call - 1) // call
    pad = ncall * call - n
    if pad:
        flat = np.concatenate([flat, np.zeros(pad, flat.dtype)])
    cols = np.concatenate(
        [flat[k * call:(k + 1) * call].reshape(call // 16, 16).T
         for k in range(ncall)], axis=1)          # [16, ncall*call/16]
    return np.concatenate([cols] * 8, axis=0)     # [128, ...]


# ---------------------------------------------------------------------------
# Host-side data preparation
# ---------------------------------------------------------------------------

def _prep(x, theta, bias, ratelog, baserate, cap, sign, conf, delay, src, dst,
          mask, n_cores):
    B, N = x.shape
    E = src.shape[0]

    src = np.asarray(src).astype(np.int64)
    dst = np.asarray(dst).astype(np.int64)
    theta = np.asarray(theta, dtype=np.float32)
    sign = np.asarray(sign, dtype=np.float32)
    conf = np.asarray(conf, dtype=np.float32)
    delay = np.asarray(delay, dtype=np.float32)
    mask8 = np.asarray(mask).astype(np.int8)

    deg = np.bincount(dst, minlength=N)
    order = np.argsort(-deg, kind="stable")
    npc = (N + n_cores - 1) // n_cores
    G = (npc + P - 1) // P
    nper = G * P                                   # nodes per core (padded)

    rank_of = np.empty(N, dtype=np.int64)
    rank_of[order] = np.arange(N)
    core_of = rank_of % n_cores                    # node -> core
    pos_of = rank_of // n_cores                    # node -> position in core

    n_pad = ((N + ROWE - 1) // ROWE) * ROWE
    nq = (n_pad + SRC_CHUNK - 1) // SRC_CHUNK     # structures
    q_of = src // SRC_CHUNK                        # edge -> structure

    # per (core, structure) in-degree
    edge_core = core_of[dst]
    edge_pos = pos_of[dst]
    degq = np.zeros((n_cores, nper, nq), dtype=np.int64)
    np.add.at(degq, (edge_core, edge_pos, q_of), 1)

    # shared-over-cores placement per structure: within each core sort
    # positions by degq desc; group windows of 128; D = max over cores.
    D = np.zeros((nq, G), dtype=np.int64)
    ordq = np.zeros((n_cores, nq, nper), dtype=np.int64)   # row j -> position
    invq = np.zeros((n_cores, nq, nper), dtype=np.int64)   # position -> row j
    for q in range(nq):
        for c in range(n_cores):
            o = np.argsort(-degq[c, :, q], kind="stable")
            ordq[c, q] = o
            invq[c, q, o] = np.arange(nper)
            dm = degq[c, o, q].reshape(G, P).max(axis=1)
            D[q] = np.maximum(D[q], dm)
    D[0] = np.maximum(D[0], 1)       # canonical layout covers all nodes
    S = np.zeros((nq, G + 1), dtype=np.int64)
    S[:, 1:] = np.cumsum(D, axis=1)
    F = S[:, -1]                                   # cols per structure
    Gact = np.array([int((D[q] > 0).sum()) for q in range(nq)])

    # --- edge slot assignment ---
    # edge -> (core, structure, row=invq, occurrence within (node,structure))
    eord = np.lexsort((src, dst))                  # group by dst, then src q
    ec = edge_core[eord]
    ep = edge_pos[eord]
    eq = q_of[eord]
    # occurrence counter within (dst, q): edges sorted by (dst, q)
    key_change = np.ones(E, dtype=bool)
    key_change[1:] = (dst[eord][1:] != dst[eord][:-1]) | (eq[1:] != eq[:-1])
    run_id = np.cumsum(key_change) - 1
    run_starts = np.flatnonzero(key_change)
    occ = np.arange(E) - run_starts[run_id]

    row = invq[ec, eq, ep]                         # row index in structure
    g = row // P
    pp = row % P
    col = S[eq, g] + occ
    # slot linear index within (core, structure): i = pp + 128*col
    slot_i = pp + P * col

    # per (core, structure) arrays
    ins = []
    FT = int(F.sum())                              # total cols, all structures
    Scol = np.zeros(nq + 1, dtype=np.int64)
    Scol[1:] = np.cumsum(F)

    # params laid out [P, FT] per core (slot (q,p,col) -> [p, Scol[q]+col])
    par_shape = (n_cores, P, FT)
    thetaA = np.zeros(par_shape, np.float32)
    signA = np.zeros(par_shape, np.float32)
    confA = np.zeros(par_shape, np.float32)
    delayA = np.zeros(par_shape, np.float32)
    maskA = np.zeros(par_shape, np.int8)
    pidx = (ec, pp, Scol[eq] + col)
    thetaA[pidx] = theta[eord]
    signA[pidx] = sign[eord]
    confA[pidx] = conf[eord]
    delayA[pidx] = delay[eord]
    maskA[pidx] = mask8[eord]

    # gather index lists (wrapped) per core: concat over structures of
    # the per-structure slot-order index list (idx relative to q base)
    srcrel = (src[eord] - eq * SRC_CHUNK).astype(np.int16)
    gidx = []
    ncalls = np.zeros(nq, dtype=np.int64)
    for c in range(n_cores):
        parts = []
        for q in range(nq):
            tot = int(F[q]) * P
            a = np.zeros(tot, np.int16)
            selq = (ec == c) & (eq == q)
            a[slot_i[selq]] = srcrel[selq]
            parts.append(_wrap_idx(a, GATHER_CALL))
            ncalls[q] = (tot + GATHER_CALL - 1) // GATHER_CALL
        gidx.append(np.concatenate(parts, axis=1))
    gidx_cols = gidx[0].shape[1]

    # canonical placement = structure 0's; scatter index for structures
    # 1..nq-1: input position i = p + 128*g -> canonical slot p0*G + g0
    sidx = []
    for c in range(n_cores):
        parts = []
        for q in range(1, nq):
            node_pos = ordq[c, q]                  # row j -> position
            crow = invq[c, 0, node_pos]            # canonical row of node
            canon = (crow % P) * G + (crow // P)   # partition-major slot id
            a = np.zeros(nper, np.int16)
            jj = np.arange(nper)
            a[(jj % P) + P * (jj // P)] = canon.astype(np.int16)
            nact = int(Gact[q]) * P
            half = min((G // 2) * P, nact)
            parts.append(_wrap_idx(a[:half], half))
            if nact > half:
                parts.append(_wrap_idx(a[half:nact], nact - half))
        sidx.append(np.concatenate(parts, axis=1) if parts else
                    np.zeros((128, 16), np.int16))
    sidx_cols = sidx[0].shape[1]

    # node params in canonical placement [P, G]
    def node_arr(vals, fill):
        a = np.full((n_cores, P, G), fill, dtype=np.float32)
        for c in range(n_cores):
            node_pos = ordq[c, 0]                  # canonical row j -> pos
            rank = node_pos * n_cores + c          # position -> rank
            ok = rank < N
            nd = order[np.minimum(rank, N - 1)]
            v = np.where(ok, vals[nd], fill).astype(np.float32)
            a[c].reshape(-1)[(np.arange(nper) % P) * G +
                             (np.arange(nper) // P)] = np.where(
                                 ok, v, fill)
        return a

    biasA = node_arr(np.asarray(bias), 0.0)
    ratelogA = node_arr(np.asarray(ratelog), 0.0)
    baserateA = node_arr(np.asarray(baserate), 0.0)
    capA = node_arr(np.asarray(cap), 1.0)

    import ml_dtypes
    xdt = ml_dtypes.bfloat16 if XBF16 else np.float32
    xT4 = np.zeros((n_pad, XROW), xdt)
    xT4[:N, :B] = np.asarray(x, dtype=np.float32).T.astype(xdt)

    xTf = np.asarray(x, dtype=np.float32).T
    xownA = np.zeros((n_cores, P, G, B), np.float32)
    node_ids = np.zeros((n_cores, P, G), np.int64)
    for c in range(n_cores):
        node_pos = ordq[c, 0]
        rank = node_pos * n_cores + c
        ok = rank < N
        nd = np.where(ok, order[np.minimum(rank, N - 1)], -1)
        jj = np.arange(nper)
        pcol = (jj % P, jj // P)
        node_ids[c][pcol] = nd
        xownA[c][pcol[0], pcol[1], :] = np.where(
            ok[:, None], xTf[np.maximum(nd, 0), :], 0.0)

    for c in range(n_cores):
        ins.append({
            "xT4": xT4,
            "gidx": gidx[c],
            "sidx": sidx[c],
            "theta": thetaA[c],
            "sgn": signA[c],
            "conf": confA[c],
            "delay": delayA[c],
            "maskf": maskA[c],
            "bias": biasA[c],
            "ratelog": ratelogA[c],
            "baserate": baserateA[c],
            "cap": capA[c],
            "xown": xownA[c].reshape(P, G * B),
        })
    plan = dict(B=B, N=N, G=G, nq=nq, D=D, S=S, F=F, Scol=Scol, Gact=Gact,
                n_pad=n_pad, gidx_cols=gidx_cols, sidx_cols=sidx_cols,
                node_ids=node_ids)
    return ins, plan


def _assemble(results, plan):
    B, N, G = plan["B"], plan["N"], plan["G"]
    out = np.empty((B, N), dtype=np.float32)
    for ci, res in enumerate(results):
        o = res["out"].reshape(P, G, B)
        nid = plan["node_ids"][ci]
        ok = nid >= 0
        out[:, nid[ok]] = o[ok].T
    return out


# ---------------------------------------------------------------------------
# Device kernel
# ---------------------------------------------------------------------------

def _raw_dma_gather(g, out_ap, in_ap, idxs_ap, num_idxs, elem_size, elem_step,
                    queue_num):
    stride_bytes = elem_step * mybir.dt.size(in_ap.dtype)
    return g.add_instruction(
        mybir.InstDMAGatherAnt(
            name=g.bass.get_next_instruction_name(),
            ins=[*g.lower_ap_dma(in_ap, for_custom_bir_dma=True),
                 g.lower_ap(idxs_ap), g.lower_val_access(g.to_reg(num_idxs))],
            outs=[g.lower_ap(out_ap)],
            transpose=False, num_idxs=num_idxs, elem_size=elem_size,
            stride_bytes_256=stride_bytes // 256, gen_mode=0,
            single_packet=False, queue_num=queue_num,
            sbuf_tokens_per_rank=0, sbuf_free_dim_per_rank=0,
            sbuf_free_dim_pad_per_rank=0, sbuf_byte_offset=0))


def _equal_d_runs(D, g0, g1):
    runs = []
    a = g0
    while a < g1:
        b = a + 1
        while b < g1 and D[b] == D[a]:
            b += 1
        runs.append((a, b, int(D[a])))
        a = b
    return runs


def build(B, N, G, nq, D, S, F, Scol, n_pad, gidx_cols, sidx_cols,
          Gact=None, node_ids=None, enable_asserts=False, loop_r=None):
    if Gact is None:
        Gact = np.array([G] * nq)
    FT = int(Scol[-1])
    nc = bacc.Bacc("TRN2", target_bir_lowering=False, debug=False,
                   enable_asserts=enable_asserts, num_swdge_queues=4)

    XDT = mybir.dt.bfloat16 if XBF16 else F32
    xT4 = nc.dram_tensor("xT4", [n_pad, XROW], XDT, kind="ExternalInput")
    giD = nc.dram_tensor("gidx", [128, gidx_cols], I16, kind="ExternalInput")
    siD = nc.dram_tensor("sidx", [128, sidx_cols], I16, kind="ExternalInput")
    thD = nc.dram_tensor("theta", [P, FT], F32, kind="ExternalInput")
    sgD = nc.dram_tensor("sgn", [P, FT], F32, kind="ExternalInput")
    cfD = nc.dram_tensor("conf", [P, FT], F32, kind="ExternalInput")
    dlD = nc.dram_tensor("delay", [P, FT], F32, kind="ExternalInput")
    mkD = nc.dram_tensor("maskf", [P, FT], I8, kind="ExternalInput")
    biD = nc.dram_tensor("bias", [P, G], F32, kind="ExternalInput")
    rlD = nc.dram_tensor("ratelog", [P, G], F32, kind="ExternalInput")
    brD = nc.dram_tensor("baserate", [P, G], F32, kind="ExternalInput")
    cpD = nc.dram_tensor("cap", [P, G], F32, kind="ExternalInput")
    xoD = nc.dram_tensor("xown", [P, G * B], F32, kind="ExternalInput")
    outD = nc.dram_tensor("out", [P, G * B], F32, kind="ExternalOutput")
    # partial-agg merge buffers (zero-initialized by the runtime)
    pagg = [nc.dram_tensor(f"pagg{q}", [G * P, ROWE], F32,
                           kind="ExternalOutput") for q in range(1, nq)]

    Tanh = mybir.ActivationFunctionType.Tanh
    Exp = mybir.ActivationFunctionType.Exp

    qrr = [0]

    def next_q():
        qrr[0] = (qrr[0] + 1) % 4
        return qrr[0]

    import contextlib
    with tile.TileContext(nc) as tc:
        with (
            tc.tile_pool(name="persist", bufs=1) as ppool,
            tc.tile_pool(name="work", bufs=2) as wp,
            tc.tile_pool(name="msgs", bufs=2) as mp,
        ):
          with (tc.For_i(0, loop_r, 1) if loop_r else
                contextlib.nullcontext()):
            agg0 = ppool.tile([P, G * B], F32, tag="agg0")
            aggq_tiles = []

            qorder = list(range(1, nq)) + [0]
            gidx_bases = np.zeros(nq + 1, dtype=np.int64)
            sidx_bases = np.zeros(nq, dtype=np.int64)
            sb = 0
            for q in range(nq):
                tot = int(F[q]) * P
                ncall = (tot + GATHER_CALL - 1) // GATHER_CALL if tot else 0
                gidx_bases[q + 1] = gidx_bases[q] + ncall * (GATHER_CALL // 16)
                if q >= 1:
                    sidx_bases[q] = sb
                    nact = int(Gact[q]) * P
                    sb += (nact + 15) // 16
            for q in qorder:
                Fq = int(F[q])
                TOTq = Fq * P
                if TOTq == 0:
                    continue
                ncall = (TOTq + GATHER_CALL - 1) // GATHER_CALL
                icols = ncall * (GATHER_CALL // 16)
                gidx_t = wp.tile([128, icols], I16, tag="gidx")
                gb = int(gidx_bases[q])
                nc.sync.dma_start(
                    out=gidx_t[:],
                    in_=giD[:, gb:gb + icols])

                th = wp.tile([P, Fq], F32, tag="th")
                sg = wp.tile([P, Fq], F32, tag="sg")
                cf = wp.tile([P, Fq], F32, tag="cf")
                dl = wp.tile([P, Fq], F32, tag="dl")
                mk = wp.tile([P, Fq], I8, tag="mk")
                c0, c1 = int(Scol[q]), int(Scol[q + 1])
                nc.sync.dma_start(out=th[:], in_=thD[:, c0:c1])
                nc.sync.dma_start(out=sg[:], in_=sgD[:, c0:c1])
                nc.sync.dma_start(out=cf[:], in_=cfD[:, c0:c1])
                nc.sync.dma_start(out=dl[:], in_=dlD[:, c0:c1])
                nc.sync.dma_start(out=mk[:], in_=mkD[:, c0:c1])

                t = wp.tile([P, Fq], F32, tag="t")
                w = wp.tile([P, Fq], F32, tag="w")
                nc.scalar.activation(t[:], th[:], Tanh)
                nc.vector.tensor_mul(w[:], sg[:], cf[:])
                nc.vector.copy_predicated(w[:], mk[:], t[:])
                nc.vector.tensor_mul(w[:], w[:], dl[:])

                msgs = mp.tile([P, Fq * B], F32, tag="msgs")
                m3 = msgs[:].rearrange("p (s b) -> p s b", b=B)
                base = q * SRC_CHUNK
                in_ap = xT4[base:min(base + SRC_CHUNK, n_pad), :B]
                if XBF16:
                    msgsr = mp.tile([P, Fq * B], XDT, tag="msgsr")
                    gdst = msgsr[:].rearrange("p (s b) -> p s b", b=B)
                else:
                    gdst = m3
                for k in range(ncall):
                    i0 = k * GATHER_CALL
                    ni = min(GATHER_CALL, TOTq - i0)
                    _raw_dma_gather(
                        nc.gpsimd,
                        gdst[:, i0 // P:(i0 + ni) // P, :],
                        in_ap,
                        gidx_t[:, k * (GATHER_CALL // 16):
                               k * (GATHER_CALL // 16) + (ni + 15) // 16],
                        ni, B, XROW, next_q())

                wb = w[:].unsqueeze(-1).to_broadcast([P, Fq, B])
                if XBF16:
                    nc.vector.tensor_tensor(out=m3, in0=gdst, in1=wb,
                                            op=mybir.AluOpType.mult)
                else:
                    nc.vector.tensor_mul(m3, m3, wb)

                if q == 0:
                    aggt = agg0
                else:
                    aggt = wp.tile([P, G * B], F32, tag="aggq")
                    aggq_tiles.append(aggt)
                for (ga, gb2, d) in _equal_d_runs(D[q], 0, int(Gact[q])):
                    if d == 0:
                        continue
                    src_ap = (m3[:, int(S[q, ga]):int(S[q, gb2]), :]
                              .rearrange("p (n d) b -> p n b d", d=d))
                    dst_ap = aggt[:, ga * B:gb2 * B].rearrange(
                        "p (n b) -> p n b", b=B)
                    nc.vector.tensor_reduce(
                        dst_ap, src_ap, axis=mybir.AxisListType.X,
                        op=mybir.AluOpType.add)

                if q > 0:
                    # scatter active rows into canonical order through HBM
                    a3 = aggt[:].rearrange("p (g b) -> p g b", b=B)
                    nact = int(Gact[q]) * P
                    half = min((G // 2) * P, nact)
                    sbase = int(sidx_bases[q])
                    nc.gpsimd.dma_scatter_add(
                        pagg[q - 1][:, :B], a3[:, :half // P, :],
                        _slice_idx(wp, nc, siD, sbase, half),
                        half, half, B, elem_step=ROWE,
                        single_packet=False, queue_num=next_q())
                    if nact > half:
                        nc.gpsimd.dma_scatter_add(
                            pagg[q - 1][:, :B],
                            a3[:, half // P:nact // P, :],
                            _slice_idx(wp, nc, siD, sbase + half // 16,
                                       nact - half),
                            nact - half, nact - half, B, elem_step=ROWE,
                            single_packet=False, queue_num=next_q())

            # ---- merge + ODE epilogue ----
            rdb = []
            for q in range(1, nq):
                if int(Gact[q]) == 0:
                    continue
                rt = ppool.tile([P, G * B], F32, tag=f"rdb{q}")
                nc.sync.dma_start(
                    out=rt[:].rearrange("p (g b) -> p g b", b=B),
                    in_=pagg[q - 1][:, :B].rearrange(
                        "(p g) b -> p g b", p=P))
                rdb.append(rt)
            for rt in rdb:
                nc.vector.tensor_add(agg0[:], agg0[:], rt[:])

            bi = ppool.tile([P, G], F32, tag="bi")
            rl = ppool.tile([P, G], F32, tag="rl")
            br = ppool.tile([P, G], F32, tag="br")
            cp = ppool.tile([P, G], F32, tag="cp")
            xo = ppool.tile([P, G * B], F32, tag="xo")
            nc.sync.dma_start(out=bi[:], in_=biD[:, :])
            nc.sync.dma_start(out=rl[:], in_=rlD[:, :])
            nc.sync.dma_start(out=br[:], in_=brD[:, :])
            nc.sync.dma_start(out=cp[:], in_=cpD[:, :])
            nc.sync.dma_start(out=xo[:], in_=xoD[:, :])

            rate = ppool.tile([P, G], F32, tag="rate")
            nc.scalar.activation(rate[:], rl[:], Exp)
            nc.vector.tensor_mul(rate[:], rate[:], br[:])

            a3 = agg0[:].rearrange("p (g b) -> p g b", b=B)
            bib = bi[:].unsqueeze(-1).to_broadcast([P, G, B])
            cpb = cp[:].unsqueeze(-1).to_broadcast([P, G, B])
            rateb = rate[:].unsqueeze(-1).to_broadcast([P, G, B])

            dr = ppool.tile([P, G * B], F32, tag="dr")
            d3 = dr[:].rearrange("p (g b) -> p g b", b=B)
            nc.vector.tensor_add(d3, a3, bib)
            nc.scalar.activation(dr[:], dr[:], Tanh)
            nc.vector.tensor_mul(d3, d3, cpb)
            nc.vector.tensor_tensor(out=dr[:], in0=dr[:], in1=xo[:],
                                    op=mybir.AluOpType.subtract)
            nc.vector.tensor_mul(d3, d3, rateb)
            nc.vector.tensor_scalar_mul(dr[:], dr[:], float(DT))
            nc.vector.tensor_add(dr[:], dr[:], xo[:])
            nc.vector.tensor_scalar_max(dr[:], dr[:], 0.0)
            nc.vector.tensor_tensor(out=d3, in0=d3, in1=cpb,
                                    op=mybir.AluOpType.min)
            nc.sync.dma_start(out=outD[:, :], in_=dr[:])

    nc.compile()
    return nc


def _slice_idx(wp, nc, siD, col0, n):
    t = wp.tile([128, (n + 15) // 16], I16, tag="sidx")
    nc.sync.dma_start(out=t[:], in_=siD[:, col0:col0 + (n + 15) // 16])
    return t[:]


# ---------------------------------------------------------------------------
# Entry point
# ---------------------------------------------------------------------------

def kernel(x, theta_graph, node_bias, rate_log_scale, base_rate, capacity,
           sign_prior, conf_scale, delay_scale, src_index, dst_index,
           learn_mask):
    ins, plan = _prep(x, theta_graph, node_bias, rate_log_scale, base_rate,
                      capacity, sign_prior, conf_scale, delay_scale,
                      src_index, dst_index, learn_mask, NCORES)
    nc = build(plan["B"], plan["N"], plan["G"], plan["nq"], plan["D"],
               plan["S"], plan["F"], plan["Scol"], plan["n_pad"],
               plan["gidx_cols"], plan["sidx_cols"], Gact=plan["Gact"])
    res = run_bass_kernel_spmd(nc, ins, core_ids=list(range(NCORES)))
    return _assemble(res.results, plan)



# revision 4
# speedup vs baseline: 1.0105x; 1.0105x over previous
"""Trainium2 Bass kernel for DifferentiableSupergraphDynamics.

Computation:
    edge_w = where(learn_mask, tanh(theta), sign*conf) * delay      [E]
    msgs   = x[:, src] * edge_w                                     [B, E]
    agg    = scatter_add(msgs -> dst)                               [B, N]
    rate   = base_rate * exp(rate_log_scale)                        [N]
    drive  = tanh(agg + bias)
    x_next = clip(x + DT * rate * (drive*cap - x), 0, cap)

Sharding: destination nodes are dealt round-robin (by total in-degree
rank) across the 8 cores; every edge lives on its destination's core, so
no cross-core collective is needed.

Per-core edge phase: edges are split into 4 "structures" by source-node
range (32768 rows each, so dma_gather's int16 indices can address the x
table). Each structure is a padded CSR over the core's nodes sorted by
that structure's in-degree: node groups of 128 partitions padded to the
group max degree D. Structures are processed as group-aligned chunks of
<= 8192 slots; each chunk is one SWDGE dma_gather call. Calls round-robin
the 4 SWDGE queues; with the descriptor ring doubled (32KB carveout) a
queue's next generation overlaps the previous call's drain, keeping all
8 Q7 cores generating descriptors continuously (~66us per 8192-desc call
per queue). Per-chunk Vector work (weight multiply + strided
tensor_reduce) and the per-structure merge scatter-adds (into canonical
node order via HBM) ride under the SWDGE generation critical path.
"""

import numpy as np

import concourse.bass as bass
import concourse.bacc as bacc
import concourse.mybir as mybir
import concourse.tile as tile
from concourse.bass_utils import run_bass_kernel_spmd

P = 128
NCORES = 8
DT = 0.1
SRC_CHUNK = 32768          # dma_gather int16 index reach
CALL_SLOTS = 8192          # max slots per gather call (<= ring capacity)
CALL_COLS = CALL_SLOTS // P
RING_BYTES = 32768         # SWDGE descriptor carveout (2 calls in flight)
ROWE = 64                  # x-table row stride: 256B (dma_gather req)

F32 = mybir.dt.float32
I16 = mybir.dt.int16
I8 = mybir.dt.int8


def _wrap_idx(flat):
    """SWDGE wrapped index layout for one call: index j at [j%16, j//16],
    replicated to 128 partitions (each queue's Q7 pair reads its own
    16-partition stripe)."""
    n = len(flat)
    assert n % 16 == 0
    cols = flat.reshape(n // 16, 16).T            # [16, n/16]
    return np.concatenate([cols] * 8, axis=0)     # [128, n/16]


# ---------------------------------------------------------------------------
# Host-side data preparation
# ---------------------------------------------------------------------------

def _chunk_groups(D_q, gact):
    """Split active groups [0, gact) into runs with sum(D) <= CALL_COLS."""
    chunks = []
    g = 0
    while g < gact:
        g2 = g
        tot = 0
        while g2 < gact and tot + D_q[g2] <= CALL_COLS:
            tot += D_q[g2]
            g2 += 1
        assert g2 > g
        chunks.append((g, g2, int(tot)))
        g = g2
    return chunks


def _prep(x, theta, bias, ratelog, baserate, cap, sign, conf, delay, src, dst,
          mask, n_cores):
    B, N = x.shape
    E = src.shape[0]

    src = np.asarray(src).astype(np.int64)
    dst = np.asarray(dst).astype(np.int64)
    theta = np.asarray(theta, dtype=np.float32)
    sign = np.asarray(sign, dtype=np.float32)
    conf = np.asarray(conf, dtype=np.float32)
    delay = np.asarray(delay, dtype=np.float32)
    mask8 = np.asarray(mask).astype(np.int8)

    deg = np.bincount(dst, minlength=N)
    order = np.argsort(-deg, kind="stable")
    npc = (N + n_cores - 1) // n_cores
    G = (npc + P - 1) // P
    nper = G * P                                   # nodes per core (padded)

    rank_of = np.empty(N, dtype=np.int64)
    rank_of[order] = np.arange(N)
    core_of = rank_of % n_cores                    # node -> core
    pos_of = rank_of // n_cores                    # node -> position in core

    n_pad = ((N + ROWE - 1) // ROWE) * ROWE
    nq = (n_pad + SRC_CHUNK - 1) // SRC_CHUNK     # structures
    q_of = src // SRC_CHUNK                        # edge -> structure

    # per (core, structure) in-degree
    edge_core = core_of[dst]
    edge_pos = pos_of[dst]
    degq = np.zeros((n_cores, nper, nq), dtype=np.int64)
    np.add.at(degq, (edge_core, edge_pos, q_of), 1)

    # shared-over-cores placement per structure: within each core sort
    # positions by degq desc; group windows of 128; D = max over cores.
    D = np.zeros((nq, G), dtype=np.int64)
    ordq = np.zeros((n_cores, nq, nper), dtype=np.int64)   # row j -> position
    invq = np.zeros((n_cores, nq, nper), dtype=np.int64)   # position -> row j
    for q in range(nq):
        for c in range(n_cores):
            o = np.argsort(-degq[c, :, q], kind="stable")
            ordq[c, q] = o
            invq[c, q, o] = np.arange(nper)
            dm = degq[c, o, q].reshape(G, P).max(axis=1)
            D[q] = np.maximum(D[q], dm)
    D[0] = np.maximum(D[0], 1)       # canonical layout covers all nodes
    S = np.zeros((nq, G + 1), dtype=np.int64)
    S[:, 1:] = np.cumsum(D, axis=1)
    F = S[:, -1]                                   # cols per structure
    Gact = np.array([int((D[q] > 0).sum()) for q in range(nq)])

    # --- edge slot assignment ---
    eord = np.lexsort((src, dst))
    ec = edge_core[eord]
    ep = edge_pos[eord]
    eq = q_of[eord]
    key_change = np.ones(E, dtype=bool)
    key_change[1:] = (dst[eord][1:] != dst[eord][:-1]) | (eq[1:] != eq[:-1])
    run_id = np.cumsum(key_change) - 1
    run_starts = np.flatnonzero(key_change)
    occ = np.arange(E) - run_starts[run_id]

    row = invq[ec, eq, ep]                         # row index in structure
    g = row // P
    pp = row % P
    col = S[eq, g] + occ
    slot_i = pp + P * col                          # slot within (core, struct)

    FT = int(F.sum())
    Scol = np.zeros(nq + 1, dtype=np.int64)
    Scol[1:] = np.cumsum(F)

    # edge params laid out [P, FT] per core (slot (q,p,col) -> [p,Scol[q]+col])
    par_shape = (n_cores, P, FT)
    thetaA = np.zeros(par_shape, np.float32)
    signA = np.zeros(par_shape, np.float32)
    confA = np.zeros(par_shape, np.float32)
    delayA = np.zeros(par_shape, np.float32)
    maskA = np.zeros(par_shape, np.int8)
    pidx = (ec, pp, Scol[eq] + col)
    thetaA[pidx] = theta[eord]
    signA[pidx] = sign[eord]
    confA[pidx] = conf[eord]
    delayA[pidx] = delay[eord]
    maskA[pidx] = mask8[eord]

    # --- chunk plans (shared across cores: D is shared) ---
    chunks = [_chunk_groups(D[q], int(Gact[q])) for q in range(nq)]

    # emission schedule: gathers for structures in qorder, with each
    # structure's merge-scatter chunks emitted after the NEXT structure's
    # gathers (so their reduce deps are long satisfied at dispatch).
    # qorder = [1, 2, 3, 0]; scatter(1) after G2, scatter(3) after G3... etc.
    sched = []                                     # (kind, q, chunk_idx)
    sched += [("g", 1, i) for i in range(len(chunks[1]))]
    sched += [("g", 2, i) for i in range(len(chunks[2]))]
    sched += [("s", 1, i) for i in range(len(chunks[1]))]
    sched += [("g", 3, i) for i in range(len(chunks[3]))]
    sched += [("s", 3, i) for i in range(len(chunks[3]))]
    sched += [("s", 2, i) for i in range(len(chunks[2]))]
    sched += [("g", 0, i) for i in range(len(chunks[0]))]

    # --- per-core gather index blobs, per (structure, chunk) ---
    # full per-structure slot->srcrel map, then slice per chunk
    srcrel = (src[eord] - eq * SRC_CHUNK).astype(np.int16)
    gcol0 = {}                                     # (q, ci) -> gidx col offset
    gidx_parts = [[] for _ in range(n_cores)]
    colp = 0
    for q in range(nq):
        tots = int(F[q]) * P
        amaps = []
        for c in range(n_cores):
            a = np.zeros(tots, np.int16)
            selq = (ec == c) & (eq == q)
            a[slot_i[selq]] = srcrel[selq]
            amaps.append(a)
        for ci, (g0, g1, cols) in enumerate(chunks[q]):
            c0, c1 = int(S[q, g0]), int(S[q, g1])
            gcol0[(q, ci)] = colp
            for c in range(n_cores):
                gidx_parts[c].append(_wrap_idx(amaps[c][c0 * P:c1 * P]))
            colp += (c1 - c0) * P // 16
    gidx = [np.concatenate(p, axis=1) for p in gidx_parts]
    gidx_cols = gidx[0].shape[1]

    # --- per-core scatter index blobs (canonical slot ids), per chunk ---
    scol0 = {}
    sidx_parts = [[] for _ in range(n_cores)]
    colp = 0
    canon_of = np.zeros((n_cores, nq, nper), np.int16)
    for q in range(1, nq):
        for c in range(n_cores):
            node_pos = ordq[c, q]                  # row j -> position
            crow = invq[c, 0, node_pos]            # canonical row of node
            canon_of[c, q] = ((crow % P) * G + (crow // P)).astype(np.int16)
    for q in range(1, nq):
        for ci, (g0, g1, cols) in enumerate(chunks[q]):
            scol0[(q, ci)] = colp
            nact = (g1 - g0) * P
            for c in range(n_cores):
                jj = np.arange(g0 * P, g1 * P)
                a = np.zeros(nact, np.int16)
                a[(jj % P) + P * (jj // P - g0)] = canon_of[c, q, jj]
                sidx_parts[c].append(_wrap_idx(a))
            colp += nact // 16
    if colp == 0:
        sidx = [np.zeros((128, 16), np.int16) for _ in range(n_cores)]
        sidx_cols = 16
    else:
        sidx = [np.concatenate(p, axis=1) for p in sidx_parts]
        sidx_cols = sidx[0].shape[1]

    # node params in canonical placement [P, G]
    def node_arr(vals, fill):
        a = np.full((n_cores, P, G), fill, dtype=np.float32)
        for c in range(n_cores):
            node_pos = ordq[c, 0]
            rank = node_pos * n_cores + c
            ok = rank < N
            nd = order[np.minimum(rank, N - 1)]
            v = np.where(ok, vals[nd], fill).astype(np.float32)
            a[c].reshape(-1)[(np.arange(nper) % P) * G +
                             (np.arange(nper) // P)] = np.where(ok, v, fill)
        return a

    biasA = node_arr(np.asarray(bias), 0.0)
    ratelogA = node_arr(np.asarray(ratelog), 0.0)
    baserateA = node_arr(np.asarray(baserate), 0.0)
    capA = node_arr(np.asarray(cap), 1.0)

    xT4 = np.zeros((n_pad, ROWE), np.float32)
    xT4[:N, :B] = np.asarray(x, dtype=np.float32).T

    xTf = xT4[:, :B]
    xownA = np.zeros((n_cores, P, G, B), np.float32)
    node_ids = np.zeros((n_cores, P, G), np.int64)
    for c in range(n_cores):
        node_pos = ordq[c, 0]
        rank = node_pos * n_cores + c
        ok = rank < N
        nd = np.where(ok, order[np.minimum(rank, N - 1)], -1)
        jj = np.arange(nper)
        pcol = (jj % P, jj // P)
        node_ids[c][pcol] = nd
        xownA[c][pcol[0], pcol[1], :] = np.where(
            ok[:, None], xTf[np.maximum(nd, 0), :], 0.0)

    ins = []
    for c in range(n_cores):
        ins.append({
            "xT4": xT4,
            "gidx": gidx[c],
            "sidx": sidx[c],
            "theta": thetaA[c],
            "sgn": signA[c],
            "conf": confA[c],
            "delay": delayA[c],
            "maskf": maskA[c],
            "bias": biasA[c],
            "ratelog": ratelogA[c],
            "baserate": baserateA[c],
            "cap": capA[c],
            "xown": xownA[c].reshape(P, G * B),
        })
    plan = dict(B=B, N=N, G=G, nq=nq, D=D, S=S, F=F, Scol=Scol, Gact=Gact,
                n_pad=n_pad, gidx_cols=gidx_cols, sidx_cols=sidx_cols,
                chunks=chunks, sched=sched, gcol0=gcol0, scol0=scol0,
                node_ids=node_ids)
    return ins, plan


def _assemble(results, plan):
    B, N, G = plan["B"], plan["N"], plan["G"]
    out = np.empty((B, N), dtype=np.float32)
    for ci, res in enumerate(results):
        o = res["out"].reshape(P, G, B)
        nid = plan["node_ids"][ci]
        ok = nid >= 0
        out[:, nid[ok]] = o[ok].T
    return out


# ---------------------------------------------------------------------------
# Device kernel
# ---------------------------------------------------------------------------

def _raw_dma_gather(g, out_ap, in_ap, idxs_ap, num_idxs, elem_size, elem_step,
                    queue_num):
    stride_bytes = elem_step * mybir.dt.size(in_ap.dtype)
    return g.add_instruction(
        mybir.InstDMAGatherAnt(
            name=g.bass.get_next_instruction_name(),
            ins=[*g.lower_ap_dma(in_ap, for_custom_bir_dma=True),
                 g.lower_ap(idxs_ap), g.lower_val_access(g.to_reg(num_idxs))],
            outs=[g.lower_ap(out_ap)],
            transpose=False, num_idxs=num_idxs, elem_size=elem_size,
            stride_bytes_256=stride_bytes // 256, gen_mode=0,
            single_packet=False, queue_num=queue_num,
            sbuf_tokens_per_rank=0, sbuf_free_dim_per_rank=0,
            sbuf_free_dim_pad_per_rank=0, sbuf_byte_offset=0))


def _equal_d_runs(D, g0, g1):
    runs = []
    a = g0
    while a < g1:
        b = a + 1
        while b < g1 and D[b] == D[a]:
            b += 1
        runs.append((a, b, int(D[a])))
        a = b
    return runs


def build(plan):
    B = plan["B"]
    G = plan["G"]
    nq = plan["nq"]
    D, S, F, Scol = plan["D"], plan["S"], plan["F"], plan["Scol"]
    n_pad = plan["n_pad"]
    chunks, sched = plan["chunks"], plan["sched"]
    gcol0, scol0 = plan["gcol0"], plan["scol0"]
    FT = int(Scol[-1])

    nc = bacc.Bacc("TRN2", target_bir_lowering=False, debug=False,
                   enable_asserts=False, num_swdge_queues=4,
                   dynamic_dma_scratch_size=RING_BYTES)

    xT4 = nc.dram_tensor("xT4", [n_pad, ROWE], F32, kind="ExternalInput")
    giD = nc.dram_tensor("gidx", [128, plan["gidx_cols"]], I16,
                         kind="ExternalInput")
    siD = nc.dram_tensor("sidx", [128, plan["sidx_cols"]], I16,
                         kind="ExternalInput")
    thD = nc.dram_tensor("theta", [P, FT], F32, kind="ExternalInput")
    sgD = nc.dram_tensor("sgn", [P, FT], F32, kind="ExternalInput")
    cfD = nc.dram_tensor("conf", [P, FT], F32, kind="ExternalInput")
    dlD = nc.dram_tensor("delay", [P, FT], F32, kind="ExternalInput")
    mkD = nc.dram_tensor("maskf", [P, FT], I8, kind="ExternalInput")
    biD = nc.dram_tensor("bias", [P, G], F32, kind="ExternalInput")
    rlD = nc.dram_tensor("ratelog", [P, G], F32, kind="ExternalInput")
    brD = nc.dram_tensor("baserate", [P, G], F32, kind="ExternalInput")
    cpD = nc.dram_tensor("cap", [P, G], F32, kind="ExternalInput")
    xoD = nc.dram_tensor("xown", [P, G * B], F32, kind="ExternalInput")
    outD = nc.dram_tensor("out", [P, G * B], F32, kind="ExternalOutput")
    # partial-agg merge buffers (zero-initialized by the runtime)
    pagg = [nc.dram_tensor(f"pagg{q}", [G * P, ROWE], F32,
                           kind="ExternalOutput") for q in range(1, nq)]

    Tanh = mybir.ActivationFunctionType.Tanh
    Exp = mybir.ActivationFunctionType.Exp

    with tile.TileContext(nc) as tc:
        with tc.tile_pool(name="persist", bufs=1) as pp:
            gidx_t = pp.tile([128, plan["gidx_cols"]], I16, tag="gidx")
            nc.sync.dma_start(out=gidx_t[:], in_=giD[:, :])
            sidx_t = pp.tile([128, plan["sidx_cols"]], I16, tag="sidx")
            nc.sync.dma_start(out=sidx_t[:], in_=siD[:, :])

            bi = pp.tile([P, G], F32, tag="bi")
            rl = pp.tile([P, G], F32, tag="rl")
            br = pp.tile([P, G], F32, tag="br")
            cp = pp.tile([P, G], F32, tag="cp")
            xo = pp.tile([P, G * B], F32, tag="xo")
            nc.sync.dma_start(out=bi[:], in_=biD[:, :])
            nc.sync.dma_start(out=rl[:], in_=rlD[:, :])
            nc.sync.dma_start(out=br[:], in_=brD[:, :])
            nc.sync.dma_start(out=cp[:], in_=cpD[:, :])
            nc.sync.dma_start(out=xo[:], in_=xoD[:, :])

            # ---- edge weights, computed once ----
            w = pp.tile([P, FT], F32, tag="w")
            with tc.tile_pool(name="wprep", bufs=1) as wp:
                th = wp.tile([P, FT], F32, tag="th")
                sg = wp.tile([P, FT], F32, tag="sg")
                cf = wp.tile([P, FT], F32, tag="cf")
                dl = wp.tile([P, FT], F32, tag="dl")
                mk = wp.tile([P, FT], I8, tag="mk")
                nc.scalar.dma_start(out=th[:], in_=thD[:, :])
                nc.scalar.dma_start(out=sg[:], in_=sgD[:, :])
                nc.scalar.dma_start(out=cf[:], in_=cfD[:, :])
                nc.scalar.dma_start(out=dl[:], in_=dlD[:, :])
                nc.scalar.dma_start(out=mk[:], in_=mkD[:, :])
                t = wp.tile([P, FT], F32, tag="t")
                nc.scalar.activation(t[:], th[:], Tanh)
                nc.vector.tensor_mul(w[:], sg[:], cf[:])
                nc.vector.copy_predicated(w[:], mk[:], t[:])
                nc.vector.tensor_mul(w[:], w[:], dl[:])

            agg0 = pp.tile([P, G * B], F32, tag="agg0")
            aggq = {}
            for q in range(1, nq):
                aggq[q] = pp.tile([P, G * B], F32, tag=f"agg{q}",
                                  name=f"aggq{q}")

            qrr = [0]

            def next_q():
                r = qrr[0]
                qrr[0] = (r + 1) % 4
                return r

            with tc.tile_pool(name="msgs", bufs=6) as mp:
                for kind, q, ci in sched:
                    g0, g1, cols = chunks[q][ci]
                    if kind == "g":
                        slots = cols * P
                        m = mp.tile([P, CALL_COLS * B], F32, tag="m")
                        m3 = m[:, :cols * B].rearrange(
                            "p (s b) -> p s b", b=B)
                        base = q * SRC_CHUNK
                        in_ap = xT4[base:min(base + SRC_CHUNK, n_pad), :B]
                        gb = gcol0[(q, ci)]
                        _raw_dma_gather(
                            nc.gpsimd, m3, in_ap,
                            gidx_t[:, gb:gb + slots // 16],
                            slots, B, ROWE, next_q())
                        # weight multiply (chunk slice of w, global cols)
                        c0, c1 = int(S[q, g0]), int(S[q, g1])
                        w0 = int(Scol[q])
                        wb = (w[:, w0 + c0:w0 + c1].unsqueeze(-1)
                              .to_broadcast([P, cols, B]))
                        nc.vector.tensor_mul(m3, m3, wb)
                        # segmented reduce into agg tile
                        aggt = agg0 if q == 0 else aggq[q]
                        for (ga, gb2, d) in _equal_d_runs(D[q], g0, g1):
                            src_ap = (m[:, (int(S[q, ga]) - c0) * B:
                                        (int(S[q, gb2]) - c0) * B]
                                      .rearrange("p (n d b) -> p n b d",
                                                 d=d, b=B))
                            dst_ap = aggt[:, ga * B:gb2 * B].rearrange(
                                "p (n b) -> p n b", b=B)
                            nc.vector.tensor_reduce(
                                dst_ap, src_ap, axis=mybir.AxisListType.X,
                                op=mybir.AluOpType.add)
                    else:
                        # merge scatter-add of aggq chunk into canonical order
                        nact = (g1 - g0) * P
                        a3 = aggq[q][:, g0 * B:g1 * B].rearrange(
                            "p (g b) -> p g b", b=B)
                        sb = scol0[(q, ci)]
                        nc.gpsimd.dma_scatter_add(
                            pagg[q - 1][:, :B], a3,
                            sidx_t[:, sb:sb + nact // 16],
                            nact, nact, B, elem_step=ROWE,
                            single_packet=False, queue_num=next_q())

            # ---- merge + ODE epilogue ----
            rdb = []
            for q in range(1, nq):
                rt = pp.tile([P, G * B], F32, tag=f"rdb{q}")
                nc.sync.dma_start(
                    out=rt[:].rearrange("p (g b) -> p g b", b=B),
                    in_=pagg[q - 1][:, :B].rearrange(
                        "(p g) b -> p g b", p=P))
                rdb.append(rt)
            for rt in rdb:
                nc.vector.tensor_add(agg0[:], agg0[:], rt[:])

            rate = pp.tile([P, G], F32, tag="rate")
            nc.scalar.activation(rate[:], rl[:], Exp)
            nc.vector.tensor_mul(rate[:], rate[:], br[:])

            a3 = agg0[:].rearrange("p (g b) -> p g b", b=B)
            bib = bi[:].unsqueeze(-1).to_broadcast([P, G, B])
            cpb = cp[:].unsqueeze(-1).to_broadcast([P, G, B])
            rateb = rate[:].unsqueeze(-1).to_broadcast([P, G, B])

            dr = pp.tile([P, G * B], F32, tag="dr")
            d3 = dr[:].rearrange("p (g b) -> p g b", b=B)
            nc.vector.tensor_add(d3, a3, bib)
            nc.scalar.activation(dr[:], dr[:], Tanh)
            nc.vector.tensor_mul(d3, d3, cpb)
            nc.vector.tensor_tensor(out=dr[:], in0=dr[:], in1=xo[:],
                                    op=mybir.AluOpType.subtract)
            nc.vector.tensor_mul(d3, d3, rateb)
            nc.vector.tensor_scalar_mul(dr[:], dr[:], float(DT))
            nc.vector.tensor_add(dr[:], dr[:], xo[:])
            nc.vector.tensor_scalar_max(dr[:], dr[:], 0.0)
            nc.vector.tensor_tensor(out=d3, in0=d3, in1=cpb,
                                    op=mybir.AluOpType.min)
            nc.sync.dma_start(out=outD[:, :], in_=dr[:])

    nc.compile()
    return nc


# ---------------------------------------------------------------------------
# Entry point
# ---------------------------------------------------------------------------

def kernel(x, theta_graph, node_bias, rate_log_scale, base_rate, capacity,
           sign_prior, conf_scale, delay_scale, src_index, dst_index,
           learn_mask):
    ins, plan = _prep(x, theta_graph, node_bias, rate_log_scale, base_rate,
                      capacity, sign_prior, conf_scale, delay_scale,
                      src_index, dst_index, learn_mask, NCORES)
    nc = build(plan)
    res = run_bass_kernel_spmd(nc, ins, core_ids=list(range(NCORES)))
    return _assemble(res.results, plan)


# revision 5
# speedup vs baseline: 1.0709x; 1.0598x over previous
"""Trainium2 Bass kernel for DifferentiableSupergraphDynamics.

Computation:
    edge_w = where(learn_mask, tanh(theta), sign*conf) * delay      [E]
    msgs   = x[:, src] * edge_w                                     [B, E]
    agg    = scatter_add(msgs -> dst)                               [B, N]
    rate   = base_rate * exp(rate_log_scale)                        [N]
    drive  = tanh(agg + bias)
    x_next = clip(x + DT * rate * (drive*cap - x), 0, cap)

Sharding: destination nodes are dealt round-robin (by total in-degree
rank) across the 8 cores; every edge lives on its destination's core, so
no cross-core collective is needed.

Per-core edge phase: edges are split into 4 "structures" by source-node
range (32768 rows each, so dma_gather's int16 indices can address the x
table). Each structure is a padded CSR over the core's nodes sorted by
that structure's in-degree: node groups of 128 partitions padded to the
group max degree D. Structures are processed as group-aligned chunks of
<= 8192 slots; each chunk is one SWDGE dma_gather call. Calls round-robin
the 4 SWDGE queues; with the descriptor ring doubled (32KB carveout) a
queue's next generation overlaps the previous call's drain, keeping all
8 Q7 cores generating descriptors continuously (~66us per 8192-desc call
per queue). Per-chunk Vector work (weight multiply + strided
tensor_reduce) and the per-structure merge scatter-adds (into canonical
node order via HBM) ride under the SWDGE generation critical path.
"""

import numpy as np

import concourse.bass as bass
import concourse.bacc as bacc
import concourse.mybir as mybir
import concourse.tile as tile
from concourse.bass_utils import run_bass_kernel_spmd

P = 128
NCORES = 8
DT = 0.1
SRC_CHUNK = 32768          # dma_gather int16 index reach
CALL_SLOTS = 8192          # max slots per gather call (<= ring capacity)
CALL_COLS = CALL_SLOTS // P
RING_BYTES = 32768         # SWDGE descriptor carveout (2 calls in flight)
ROWE = 64                  # x-table row stride: 256B (dma_gather req)

F32 = mybir.dt.float32
I16 = mybir.dt.int16
I8 = mybir.dt.int8


def _wrap_idx(flat):
    """SWDGE wrapped index layout for one call: index j at [j%16, j//16],
    replicated to 128 partitions (each queue's Q7 pair reads its own
    16-partition stripe)."""
    n = len(flat)
    assert n % 16 == 0
    cols = flat.reshape(n // 16, 16).T            # [16, n/16]
    return np.concatenate([cols] * 8, axis=0)     # [128, n/16]


# ---------------------------------------------------------------------------
# Host-side data preparation
# ---------------------------------------------------------------------------

def _chunk_groups(D_q, gact):
    """Split active groups [0, gact) into runs with sum(D) <= CALL_COLS."""
    chunks = []
    g = 0
    while g < gact:
        g2 = g
        tot = 0
        while g2 < gact and tot + D_q[g2] <= CALL_COLS:
            tot += D_q[g2]
            g2 += 1
        assert g2 > g
        chunks.append((g, g2, int(tot)))
        g = g2
    return chunks


def _prep(x, theta, bias, ratelog, baserate, cap, sign, conf, delay, src, dst,
          mask, n_cores):
    B, N = x.shape
    E = src.shape[0]

    src = np.asarray(src).astype(np.int64)
    dst = np.asarray(dst).astype(np.int64)
    theta = np.asarray(theta, dtype=np.float32)
    sign = np.asarray(sign, dtype=np.float32)
    conf = np.asarray(conf, dtype=np.float32)
    delay = np.asarray(delay, dtype=np.float32)
    mask8 = np.asarray(mask).astype(np.int8)

    deg = np.bincount(dst, minlength=N)
    order = np.argsort(-deg, kind="stable")
    npc = (N + n_cores - 1) // n_cores
    G = (npc + P - 1) // P
    nper = G * P                                   # nodes per core (padded)

    rank_of = np.empty(N, dtype=np.int64)
    rank_of[order] = np.arange(N)
    core_of = rank_of % n_cores                    # node -> core
    pos_of = rank_of // n_cores                    # node -> position in core

    n_pad = ((N + ROWE - 1) // ROWE) * ROWE
    nq = (n_pad + SRC_CHUNK - 1) // SRC_CHUNK     # structures
    q_of = src // SRC_CHUNK                        # edge -> structure

    # per (core, structure) in-degree
    edge_core = core_of[dst]
    edge_pos = pos_of[dst]
    degq = np.zeros((n_cores, nper, nq), dtype=np.int64)
    np.add.at(degq, (edge_core, edge_pos, q_of), 1)

    # shared-over-cores placement per structure: within each core sort
    # positions by degq desc; group windows of 128; D = max over cores.
    D = np.zeros((nq, G), dtype=np.int64)
    ordq = np.zeros((n_cores, nq, nper), dtype=np.int64)   # row j -> position
    invq = np.zeros((n_cores, nq, nper), dtype=np.int64)   # position -> row j
    for q in range(nq):
        for c in range(n_cores):
            o = np.argsort(-degq[c, :, q], kind="stable")
            ordq[c, q] = o
            invq[c, q, o] = np.arange(nper)
            dm = degq[c, o, q].reshape(G, P).max(axis=1)
            D[q] = np.maximum(D[q], dm)
    D[0] = np.maximum(D[0], 1)       # canonical layout covers all nodes
    S = np.zeros((nq, G + 1), dtype=np.int64)
    S[:, 1:] = np.cumsum(D, axis=1)
    F = S[:, -1]                                   # cols per structure
    Gact = np.array([int((D[q] > 0).sum()) for q in range(nq)])

    # --- edge slot assignment ---
    eord = np.lexsort((src, dst))
    ec = edge_core[eord]
    ep = edge_pos[eord]
    eq = q_of[eord]
    key_change = np.ones(E, dtype=bool)
    key_change[1:] = (dst[eord][1:] != dst[eord][:-1]) | (eq[1:] != eq[:-1])
    run_id = np.cumsum(key_change) - 1
    run_starts = np.flatnonzero(key_change)
    occ = np.arange(E) - run_starts[run_id]

    row = invq[ec, eq, ep]                         # row index in structure
    g = row // P
    pp = row % P
    col = S[eq, g] + occ
    slot_i = pp + P * col                          # slot within (core, struct)

    FT = int(F.sum())
    Scol = np.zeros(nq + 1, dtype=np.int64)
    Scol[1:] = np.cumsum(F)

    # edge params laid out [P, FT] per core (slot (q,p,col) -> [p,Scol[q]+col])
    par_shape = (n_cores, P, FT)
    thetaA = np.zeros(par_shape, np.float32)
    signA = np.zeros(par_shape, np.float32)
    confA = np.zeros(par_shape, np.float32)
    delayA = np.zeros(par_shape, np.float32)
    maskA = np.zeros(par_shape, np.int8)
    pidx = (ec, pp, Scol[eq] + col)
    thetaA[pidx] = theta[eord]
    signA[pidx] = sign[eord]
    confA[pidx] = conf[eord]
    delayA[pidx] = delay[eord]
    maskA[pidx] = mask8[eord]

    # --- chunk plans (shared across cores: D is shared) ---
    chunks = [_chunk_groups(D[q], int(Gact[q])) for q in range(nq)]

    # emission schedule: gathers for structures in qorder, with each
    # structure's merge-scatter chunks emitted after the NEXT structure's
    # gathers (so their reduce deps are long satisfied at dispatch).
    # qorder = [1, 2, 3, 0]; scatter(1) after G2, scatter(3) after G3... etc.
    sched = []                                     # (kind, q, chunk_idx)
    sched += [("g", 1, i) for i in range(len(chunks[1]))]
    sched += [("g", 2, i) for i in range(len(chunks[2]))]
    sched += [("s", 1, i) for i in range(len(chunks[1]))]
    sched += [("g", 3, i) for i in range(len(chunks[3]))]
    sched += [("s", 3, i) for i in range(len(chunks[3]))]
    sched += [("s", 2, i) for i in range(len(chunks[2]))]
    sched += [("g", 0, i) for i in range(len(chunks[0]))]

    # --- per-core gather index blobs, per (structure, chunk) ---
    # full per-structure slot->srcrel map, then slice per chunk
    srcrel = (src[eord] - eq * SRC_CHUNK).astype(np.int16)
    gcol0 = {}                                     # (q, ci) -> gidx col offset
    gidx_parts = [[] for _ in range(n_cores)]
    colp = 0
    for q in range(nq):
        tots = int(F[q]) * P
        amaps = []
        for c in range(n_cores):
            a = np.zeros(tots, np.int16)
            selq = (ec == c) & (eq == q)
            a[slot_i[selq]] = srcrel[selq]
            amaps.append(a)
        for ci, (g0, g1, cols) in enumerate(chunks[q]):
            c0, c1 = int(S[q, g0]), int(S[q, g1])
            gcol0[(q, ci)] = colp
            for c in range(n_cores):
                gidx_parts[c].append(_wrap_idx(amaps[c][c0 * P:c1 * P]))
            colp += (c1 - c0) * P // 16
    gidx = [np.concatenate(p, axis=1) for p in gidx_parts]
    gidx_cols = gidx[0].shape[1]

    # --- per-core scatter index blobs (canonical slot ids), per chunk ---
    scol0 = {}
    sidx_parts = [[] for _ in range(n_cores)]
    colp = 0
    canon_of = np.zeros((n_cores, nq, nper), np.int16)
    for q in range(1, nq):
        for c in range(n_cores):
            node_pos = ordq[c, q]                  # row j -> position
            crow = invq[c, 0, node_pos]            # canonical row of node
            canon_of[c, q] = ((crow % P) * G + (crow // P)).astype(np.int16)
    for q in range(1, nq):
        for ci, (g0, g1, cols) in enumerate(chunks[q]):
            scol0[(q, ci)] = colp
            nact = (g1 - g0) * P
            for c in range(n_cores):
                jj = np.arange(g0 * P, g1 * P)
                a = np.zeros(nact, np.int16)
                a[(jj % P) + P * (jj // P - g0)] = canon_of[c, q, jj]
                sidx_parts[c].append(_wrap_idx(a))
            colp += nact // 16
    if colp == 0:
        sidx = [np.zeros((128, 16), np.int16) for _ in range(n_cores)]
        sidx_cols = 16
    else:
        sidx = [np.concatenate(p, axis=1) for p in sidx_parts]
        sidx_cols = sidx[0].shape[1]

    # node params in canonical placement [P, G]
    def node_arr(vals, fill):
        a = np.full((n_cores, P, G), fill, dtype=np.float32)
        for c in range(n_cores):
            node_pos = ordq[c, 0]
            rank = node_pos * n_cores + c
            ok = rank < N
            nd = order[np.minimum(rank, N - 1)]
            v = np.where(ok, vals[nd], fill).astype(np.float32)
            a[c].reshape(-1)[(np.arange(nper) % P) * G +
                             (np.arange(nper) // P)] = np.where(ok, v, fill)
        return a

    biasA = node_arr(np.asarray(bias), 0.0)
    ratelogA = node_arr(np.asarray(ratelog), 0.0)
    baserateA = node_arr(np.asarray(baserate), 0.0)
    capA = node_arr(np.asarray(cap), 1.0)

    xT4 = np.zeros((n_pad, ROWE), np.float32)
    xT4[:N, :B] = np.asarray(x, dtype=np.float32).T

    xTf = xT4[:, :B]
    xownA = np.zeros((n_cores, P, G, B), np.float32)
    node_ids = np.zeros((n_cores, P, G), np.int64)
    for c in range(n_cores):
        node_pos = ordq[c, 0]
        rank = node_pos * n_cores + c
        ok = rank < N
        nd = np.where(ok, order[np.minimum(rank, N - 1)], -1)
        jj = np.arange(nper)
        pcol = (jj % P, jj // P)
        node_ids[c][pcol] = nd
        xownA[c][pcol[0], pcol[1], :] = np.where(
            ok[:, None], xTf[np.maximum(nd, 0), :], 0.0)

    ins = []
    for c in range(n_cores):
        ins.append({
            "xT4": xT4,
            "gidx": gidx[c],
            "sidx": sidx[c],
            "theta": thetaA[c],
            "sgn": signA[c],
            "conf": confA[c],
            "delay": delayA[c],
            "maskf": maskA[c],
            "bias": biasA[c],
            "ratelog": ratelogA[c],
            "baserate": baserateA[c],
            "cap": capA[c],
            "xown": xownA[c].reshape(P, G * B),
        })
    plan = dict(B=B, N=N, G=G, nq=nq, D=D, S=S, F=F, Scol=Scol, Gact=Gact,
                n_pad=n_pad, gidx_cols=gidx_cols, sidx_cols=sidx_cols,
                chunks=chunks, sched=sched, gcol0=gcol0, scol0=scol0,
                node_ids=node_ids)
    return ins, plan


def _assemble(results, plan):
    B, N, G = plan["B"], plan["N"], plan["G"]
    out = np.empty((B, N), dtype=np.float32)
    for ci, res in enumerate(results):
        o = res["out"].reshape(P, G, B)
        nid = plan["node_ids"][ci]
        ok = nid >= 0
        out[:, nid[ok]] = o[ok].T
    return out


# ---------------------------------------------------------------------------
# Device kernel
# ---------------------------------------------------------------------------

def _raw_dma_gather(g, out_ap, in_ap, idxs_ap, num_idxs, elem_size, elem_step,
                    queue_num):
    stride_bytes = elem_step * mybir.dt.size(in_ap.dtype)
    return g.add_instruction(
        mybir.InstDMAGatherAnt(
            name=g.bass.get_next_instruction_name(),
            ins=[*g.lower_ap_dma(in_ap, for_custom_bir_dma=True),
                 g.lower_ap(idxs_ap), g.lower_val_access(g.to_reg(num_idxs))],
            outs=[g.lower_ap(out_ap)],
            transpose=False, num_idxs=num_idxs, elem_size=elem_size,
            stride_bytes_256=stride_bytes // 256, gen_mode=0,
            single_packet=False, queue_num=queue_num,
            sbuf_tokens_per_rank=0, sbuf_free_dim_per_rank=0,
            sbuf_free_dim_pad_per_rank=0, sbuf_byte_offset=0))


def _equal_d_runs(D, g0, g1):
    runs = []
    a = g0
    while a < g1:
        b = a + 1
        while b < g1 and D[b] == D[a]:
            b += 1
        runs.append((a, b, int(D[a])))
        a = b
    return runs


def build(plan):
    B = plan["B"]
    G = plan["G"]
    nq = plan["nq"]
    D, S, F, Scol = plan["D"], plan["S"], plan["F"], plan["Scol"]
    n_pad = plan["n_pad"]
    chunks, sched = plan["chunks"], plan["sched"]
    gcol0, scol0 = plan["gcol0"], plan["scol0"]
    FT = int(Scol[-1])

    nc = bacc.Bacc("TRN2", target_bir_lowering=False, debug=False,
                   enable_asserts=False, num_swdge_queues=4,
                   dynamic_dma_scratch_size=RING_BYTES)

    xT4 = nc.dram_tensor("xT4", [n_pad, ROWE], F32, kind="ExternalInput")
    giD = nc.dram_tensor("gidx", [128, plan["gidx_cols"]], I16,
                         kind="ExternalInput")
    siD = nc.dram_tensor("sidx", [128, plan["sidx_cols"]], I16,
                         kind="ExternalInput")
    thD = nc.dram_tensor("theta", [P, FT], F32, kind="ExternalInput")
    sgD = nc.dram_tensor("sgn", [P, FT], F32, kind="ExternalInput")
    cfD = nc.dram_tensor("conf", [P, FT], F32, kind="ExternalInput")
    dlD = nc.dram_tensor("delay", [P, FT], F32, kind="ExternalInput")
    mkD = nc.dram_tensor("maskf", [P, FT], I8, kind="ExternalInput")
    biD = nc.dram_tensor("bias", [P, G], F32, kind="ExternalInput")
    rlD = nc.dram_tensor("ratelog", [P, G], F32, kind="ExternalInput")
    brD = nc.dram_tensor("baserate", [P, G], F32, kind="ExternalInput")
    cpD = nc.dram_tensor("cap", [P, G], F32, kind="ExternalInput")
    xoD = nc.dram_tensor("xown", [P, G * B], F32, kind="ExternalInput")
    outD = nc.dram_tensor("out", [P, G * B], F32, kind="ExternalOutput")
    # partial-agg merge buffers (zero-initialized by the runtime)
    pagg = [nc.dram_tensor(f"pagg{q}", [G * P, ROWE], F32,
                           kind="ExternalOutput") for q in range(1, nq)]

    Tanh = mybir.ActivationFunctionType.Tanh
    Exp = mybir.ActivationFunctionType.Exp

    with tile.TileContext(nc) as tc:
        with tc.tile_pool(name="persist", bufs=1) as pp:
            gidx_t = pp.tile([128, plan["gidx_cols"]], I16, tag="gidx")
            nc.sync.dma_start(out=gidx_t[:], in_=giD[:, :])
            sidx_t = pp.tile([128, plan["sidx_cols"]], I16, tag="sidx")
            nc.sync.dma_start(out=sidx_t[:], in_=siD[:, :])

            bi = pp.tile([P, G], F32, tag="bi")
            rl = pp.tile([P, G], F32, tag="rl")
            br = pp.tile([P, G], F32, tag="br")
            cp = pp.tile([P, G], F32, tag="cp")
            xo = pp.tile([P, G * B], F32, tag="xo")
            nc.sync.dma_start(out=bi[:], in_=biD[:, :])
            nc.sync.dma_start(out=rl[:], in_=rlD[:, :])
            nc.sync.dma_start(out=br[:], in_=brD[:, :])
            nc.sync.dma_start(out=cp[:], in_=cpD[:, :])
            nc.sync.dma_start(out=xo[:], in_=xoD[:, :])

            # ---- edge weights, computed once ----
            w = pp.tile([P, FT], F32, tag="w")
            with tc.tile_pool(name="wprep", bufs=1) as wp:
                th = wp.tile([P, FT], F32, tag="th")
                sg = wp.tile([P, FT], F32, tag="sg")
                cf = wp.tile([P, FT], F32, tag="cf")
                dl = wp.tile([P, FT], F32, tag="dl")
                mk = wp.tile([P, FT], I8, tag="mk")
                nc.scalar.dma_start(out=th[:], in_=thD[:, :])
                nc.scalar.dma_start(out=sg[:], in_=sgD[:, :])
                nc.scalar.dma_start(out=cf[:], in_=cfD[:, :])
                nc.scalar.dma_start(out=dl[:], in_=dlD[:, :])
                nc.scalar.dma_start(out=mk[:], in_=mkD[:, :])
                t = wp.tile([P, FT], F32, tag="t")
                nc.scalar.activation(t[:], th[:], Tanh)
                nc.vector.tensor_mul(w[:], sg[:], cf[:])
                nc.vector.copy_predicated(w[:], mk[:], t[:])
                nc.vector.tensor_mul(w[:], w[:], dl[:])

            agg0 = pp.tile([P, G * B], F32, tag="agg0")
            aggq = {}
            for q in range(1, nq):
                aggq[q] = pp.tile([P, G * B], F32, tag=f"agg{q}",
                                  name=f"aggq{q}")

            qrr = [0]

            def next_q():
                r = qrr[0]
                qrr[0] = (r + 1) % 4
                return r

            with tc.tile_pool(name="msgs", bufs=12) as mp:
                for kind, q, ci in sched:
                    g0, g1, cols = chunks[q][ci]
                    if kind == "g":
                        slots = cols * P
                        m = mp.tile([P, CALL_COLS * B], F32, tag="m")
                        m3 = m[:, :cols * B].rearrange(
                            "p (s b) -> p s b", b=B)
                        base = q * SRC_CHUNK
                        in_ap = xT4[base:min(base + SRC_CHUNK, n_pad), :B]
                        gb = gcol0[(q, ci)]
                        _raw_dma_gather(
                            nc.gpsimd, m3, in_ap,
                            gidx_t[:, gb:gb + slots // 16],
                            slots, B, ROWE, next_q())
                        # weight multiply (chunk slice of w, global cols)
                        c0, c1 = int(S[q, g0]), int(S[q, g1])
                        w0 = int(Scol[q])
                        wb = (w[:, w0 + c0:w0 + c1].unsqueeze(-1)
                              .to_broadcast([P, cols, B]))
                        nc.vector.tensor_mul(m3, m3, wb)
                        # segmented reduce into agg tile
                        aggt = agg0 if q == 0 else aggq[q]
                        for (ga, gb2, d) in _equal_d_runs(D[q], g0, g1):
                            src_ap = (m[:, (int(S[q, ga]) - c0) * B:
                                        (int(S[q, gb2]) - c0) * B]
                                      .rearrange("p (n d b) -> p n b d",
                                                 d=d, b=B))
                            dst_ap = aggt[:, ga * B:gb2 * B].rearrange(
                                "p (n b) -> p n b", b=B)
                            nc.vector.tensor_reduce(
                                dst_ap, src_ap, axis=mybir.AxisListType.X,
                                op=mybir.AluOpType.add)
                    else:
                        # merge scatter-add of aggq chunk into canonical order
                        nact = (g1 - g0) * P
                        a3 = aggq[q][:, g0 * B:g1 * B].rearrange(
                            "p (g b) -> p g b", b=B)
                        sb = scol0[(q, ci)]
                        nc.gpsimd.dma_scatter_add(
                            pagg[q - 1][:, :B], a3,
                            sidx_t[:, sb:sb + nact // 16],
                            nact, nact, B, elem_step=ROWE,
                            single_packet=False, queue_num=next_q())

            # ---- merge + ODE epilogue ----
            rdb = []
            for q in range(1, nq):
                rt = pp.tile([P, G * B], F32, tag=f"rdb{q}")
                nc.sync.dma_start(
                    out=rt[:].rearrange("p (g b) -> p g b", b=B),
                    in_=pagg[q - 1][:, :B].rearrange(
                        "(p g) b -> p g b", p=P))
                rdb.append(rt)
            for rt in rdb:
                nc.vector.tensor_add(agg0[:], agg0[:], rt[:])

            rate = pp.tile([P, G], F32, tag="rate")
            nc.scalar.activation(rate[:], rl[:], Exp)
            nc.vector.tensor_mul(rate[:], rate[:], br[:])

            a3 = agg0[:].rearrange("p (g b) -> p g b", b=B)
            bib = bi[:].unsqueeze(-1).to_broadcast([P, G, B])
            cpb = cp[:].unsqueeze(-1).to_broadcast([P, G, B])
            rateb = rate[:].unsqueeze(-1).to_broadcast([P, G, B])

            dr = pp.tile([P, G * B], F32, tag="dr")
            d3 = dr[:].rearrange("p (g b) -> p g b", b=B)
            nc.vector.tensor_add(d3, a3, bib)
            nc.scalar.activation(dr[:], dr[:], Tanh)
            nc.vector.tensor_mul(d3, d3, cpb)
            nc.vector.tensor_tensor(out=dr[:], in0=dr[:], in1=xo[:],
                                    op=mybir.AluOpType.subtract)
            nc.vector.tensor_mul(d3, d3, rateb)
            nc.vector.tensor_scalar_mul(dr[:], dr[:], float(DT))
            nc.vector.tensor_add(dr[:], dr[:], xo[:])
            nc.vector.tensor_scalar_max(dr[:], dr[:], 0.0)
            nc.vector.tensor_tensor(out=d3, in0=d3, in1=cpb,
                                    op=mybir.AluOpType.min)
            nc.sync.dma_start(out=outD[:, :], in_=dr[:])

    nc.compile()
    return nc


# ---------------------------------------------------------------------------
# Entry point
# ---------------------------------------------------------------------------

def kernel(x, theta_graph, node_bias, rate_log_scale, base_rate, capacity,
           sign_prior, conf_scale, delay_scale, src_index, dst_index,
           learn_mask):
    ins, plan = _prep(x, theta_graph, node_bias, rate_log_scale, base_rate,
                      capacity, sign_prior, conf_scale, delay_scale,
                      src_index, dst_index, learn_mask, NCORES)
    nc = build(plan)
    res = run_bass_kernel_spmd(nc, ins, core_ids=list(range(NCORES)))
    return _assemble(res.results, plan)


# revision 15
# speedup vs baseline: 1.1186x; 1.0446x over previous
"""Trainium2 Bass kernel for DifferentiableSupergraphDynamics.

Computation:
    edge_w = where(learn_mask, tanh(theta), sign*conf) * delay      [E]
    msgs   = x[:, src] * edge_w                                     [B, E]
    agg    = scatter_add(msgs -> dst)                               [B, N]
    rate   = base_rate * exp(rate_log_scale)                        [N]
    drive  = tanh(agg + bias)
    x_next = clip(x + DT * rate * (drive*cap - x), 0, cap)

Sharding: destination nodes are dealt round-robin (by total in-degree
rank) across the 8 cores; every edge lives on its destination's core, so
no cross-core collective is needed.

Per-core edge phase: edges are split into 4 "structures" by source-node
range (32768 rows each, so dma_gather's int16 indices can address the x
table). Each structure is a padded CSR over the core's nodes sorted by
that structure's in-degree: node groups of 128 partitions padded to the
group max degree D. Structures are processed as group-aligned chunks of
<= 8192 slots; each chunk is one SWDGE dma_gather call. Calls round-robin
the 4 SWDGE queues; with the descriptor ring doubled (32KB carveout) a
queue's next generation overlaps the previous call's drain, keeping all
8 Q7 cores generating descriptors continuously (~66us per 8192-desc call
per queue). Per-chunk Vector work (weight multiply + strided
tensor_reduce) and the per-structure merge scatter-adds (into canonical
node order via HBM) ride under the SWDGE generation critical path.
"""

import numpy as np

import concourse.bass as bass
import concourse.bacc as bacc
import concourse.mybir as mybir
import concourse.tile as tile
from concourse.bass_utils import run_bass_kernel_spmd

P = 128
NCORES = 8
DT = 0.1
SRC_CHUNK = 32768          # dma_gather int16 index reach
CALL_SLOTS = 8192          # max slots per gather call (<= ring capacity)
CALL_COLS = CALL_SLOTS // P
RING_BYTES = 32768         # SWDGE descriptor carveout (2 calls in flight)
ROWE = 64                  # x-table row stride: 256B (dma_gather req)

F32 = mybir.dt.float32
I16 = mybir.dt.int16
I8 = mybir.dt.int8


def _wrap_idx(flat):
    """SWDGE wrapped index layout for one call: index j at [j%16, j//16],
    replicated to 128 partitions (each queue's Q7 pair reads its own
    16-partition stripe)."""
    n = len(flat)
    assert n % 16 == 0
    cols = flat.reshape(n // 16, 16).T            # [16, n/16]
    return np.concatenate([cols] * 8, axis=0)     # [128, n/16]


# ---------------------------------------------------------------------------
# Host-side data preparation
# ---------------------------------------------------------------------------

def _chunk_groups(D_q, gact):
    """Split active groups [0, gact) into runs with sum(D) <= CALL_COLS."""
    chunks = []
    g = 0
    while g < gact:
        g2 = g
        tot = 0
        while g2 < gact and tot + D_q[g2] <= CALL_COLS:
            tot += D_q[g2]
            g2 += 1
        assert g2 > g
        chunks.append((g, g2, int(tot)))
        g = g2
    return chunks


def _prep(x, theta, bias, ratelog, baserate, cap, sign, conf, delay, src, dst,
          mask, n_cores):
    B, N = x.shape
    E = src.shape[0]

    src = np.asarray(src).astype(np.int64)
    dst = np.asarray(dst).astype(np.int64)
    theta = np.asarray(theta, dtype=np.float32)
    sign = np.asarray(sign, dtype=np.float32)
    conf = np.asarray(conf, dtype=np.float32)
    delay = np.asarray(delay, dtype=np.float32)
    mask8 = np.asarray(mask).astype(np.int8)

    deg = np.bincount(dst, minlength=N)
    order = np.argsort(-deg, kind="stable")
    npc = (N + n_cores - 1) // n_cores
    G = (npc + P - 1) // P
    nper = G * P                                   # nodes per core (padded)

    rank_of = np.empty(N, dtype=np.int64)
    rank_of[order] = np.arange(N)
    core_of = rank_of % n_cores                    # node -> core
    pos_of = rank_of // n_cores                    # node -> position in core

    n_pad = ((N + ROWE - 1) // ROWE) * ROWE
    nq = (n_pad + SRC_CHUNK - 1) // SRC_CHUNK     # structures
    q_of = src // SRC_CHUNK                        # edge -> structure

    # per (core, structure) in-degree
    edge_core = core_of[dst]
    edge_pos = pos_of[dst]
    degq = np.zeros((n_cores, nper, nq), dtype=np.int64)
    np.add.at(degq, (edge_core, edge_pos, q_of), 1)

    # shared-over-cores placement per structure: within each core sort
    # positions by degq desc; group windows of 128; D = max over cores.
    D = np.zeros((nq, G), dtype=np.int64)
    ordq = np.zeros((n_cores, nq, nper), dtype=np.int64)   # row j -> position
    invq = np.zeros((n_cores, nq, nper), dtype=np.int64)   # position -> row j
    for q in range(nq):
        for c in range(n_cores):
            o = np.argsort(-degq[c, :, q], kind="stable")
            ordq[c, q] = o
            invq[c, q, o] = np.arange(nper)
            dm = degq[c, o, q].reshape(G, P).max(axis=1)
            D[q] = np.maximum(D[q], dm)
    D[0] = np.maximum(D[0], 1)       # canonical layout covers all nodes
    S = np.zeros((nq, G + 1), dtype=np.int64)
    S[:, 1:] = np.cumsum(D, axis=1)
    F = S[:, -1]                                   # cols per structure
    Gact = np.array([int((D[q] > 0).sum()) for q in range(nq)])

    # --- edge slot assignment ---
    eord = np.lexsort((src, dst))
    ec = edge_core[eord]
    ep = edge_pos[eord]
    eq = q_of[eord]
    key_change = np.ones(E, dtype=bool)
    key_change[1:] = (dst[eord][1:] != dst[eord][:-1]) | (eq[1:] != eq[:-1])
    run_id = np.cumsum(key_change) - 1
    run_starts = np.flatnonzero(key_change)
    occ = np.arange(E) - run_starts[run_id]

    row = invq[ec, eq, ep]                         # row index in structure
    g = row // P
    pp = row % P
    col = S[eq, g] + occ
    slot_i = pp + P * col                          # slot within (core, struct)

    FT = int(F.sum())
    Scol = np.zeros(nq + 1, dtype=np.int64)
    Scol[1:] = np.cumsum(F)

    # edge params laid out [P, FT] per core (slot (q,p,col) -> [p,Scol[q]+col])
    par_shape = (n_cores, P, FT)
    thetaA = np.zeros(par_shape, np.float32)
    signA = np.zeros(par_shape, np.float32)
    confA = np.zeros(par_shape, np.float32)
    delayA = np.zeros(par_shape, np.float32)
    maskA = np.zeros(par_shape, np.int8)
    pidx = (ec, pp, Scol[eq] + col)
    thetaA[pidx] = theta[eord]
    signA[pidx] = sign[eord]
    confA[pidx] = conf[eord]
    delayA[pidx] = delay[eord]
    maskA[pidx] = mask8[eord]

    # --- chunk plans (shared across cores: D is shared) ---
    chunks = [_chunk_groups(D[q], int(Gact[q])) for q in range(nq)]

    # --- scatter chunking: blocks of bs groups with bs | Gact (zero pad) ---
    def _scatter_blocks(gact):
        for bs in range(16, 9, -1):
            if gact % bs == 0:
                return [(g, g + bs) for g in range(0, gact, bs)]
        return [(g, min(g + 16, gact)) for g in range(0, gact, 16)]

    sblocks = {q: _scatter_blocks(int(Gact[q])) for q in range(1, nq)}

    # emission schedule: gathers for structures in qorder, with each
    # structure's merge-scatter chunks emitted after the NEXT structure's
    # gathers (so their reduce deps are long satisfied at dispatch).
    # qorder = [1, 2, 3, 0]; scatter(1) after G2, scatter(3) after G3... etc.
    sched = []                                     # (kind, q, chunk/block idx)
    sched += [("g", 1, i) for i in range(len(chunks[1]))]
    sched += [("g", 2, i) for i in range(len(chunks[2]))]
    sched += [("s", 1, i) for i in range(len(sblocks[1]))]
    sched += [("g", 3, i) for i in range(len(chunks[3]))]
    sched += [("s", 3, i) for i in range(len(sblocks[3]))]
    sched += [("s", 2, i) for i in range(len(sblocks[2]))]
    sched += [("g", 0, i) for i in range(len(chunks[0]))]

    # --- per-core gather index blobs, per (structure, chunk) ---
    # full per-structure slot->srcrel map, then slice per chunk
    srcrel = (src[eord] - eq * SRC_CHUNK).astype(np.int16)
    gcol0 = {}                                     # (q, ci) -> gidx col offset
    gidx_parts = [[] for _ in range(n_cores)]
    colp = 0
    for q in range(nq):
        tots = int(F[q]) * P
        amaps = []
        for c in range(n_cores):
            a = np.zeros(tots, np.int16)
            selq = (ec == c) & (eq == q)
            a[slot_i[selq]] = srcrel[selq]
            amaps.append(a)
        for ci, (g0, g1, cols) in enumerate(chunks[q]):
            c0, c1 = int(S[q, g0]), int(S[q, g1])
            gcol0[(q, ci)] = colp
            for c in range(n_cores):
                gidx_parts[c].append(_wrap_idx(amaps[c][c0 * P:c1 * P]))
            colp += (c1 - c0) * P // 16
    gidx = [np.concatenate(p, axis=1) for p in gidx_parts]
    gidx_cols = gidx[0].shape[1]

    # --- per-core scatter index blobs (canonical slot ids), per block ---
    scol0 = {}
    sidx_parts = [[] for _ in range(n_cores)]
    colp = 0
    canon_of = np.zeros((n_cores, nq, nper), np.int16)
    for q in range(1, nq):
        for c in range(n_cores):
            node_pos = ordq[c, q]                  # row j -> position
            crow = invq[c, 0, node_pos]            # canonical row of node
            canon_of[c, q] = ((crow % P) * G + (crow // P)).astype(np.int16)
    for q in range(1, nq):
        for ci, (g0, g1) in enumerate(sblocks[q]):
            scol0[(q, ci)] = colp
            nact = (g1 - g0) * P
            for c in range(n_cores):
                jj = np.arange(g0 * P, g1 * P)
                a = np.zeros(nact, np.int16)
                a[(jj % P) + P * (jj // P - g0)] = canon_of[c, q, jj]
                sidx_parts[c].append(_wrap_idx(a))
            colp += nact // 16
    if colp == 0:
        sidx = [np.zeros((128, 16), np.int16) for _ in range(n_cores)]
        sidx_cols = 16
    else:
        sidx = [np.concatenate(p, axis=1) for p in sidx_parts]
        sidx_cols = sidx[0].shape[1]

    # node params in canonical placement [P, G]
    def node_arr(vals, fill):
        a = np.full((n_cores, P, G), fill, dtype=np.float32)
        for c in range(n_cores):
            node_pos = ordq[c, 0]
            rank = node_pos * n_cores + c
            ok = rank < N
            nd = order[np.minimum(rank, N - 1)]
            v = np.where(ok, vals[nd], fill).astype(np.float32)
            a[c].reshape(-1)[(np.arange(nper) % P) * G +
                             (np.arange(nper) // P)] = np.where(ok, v, fill)
        return a

    biasA = node_arr(np.asarray(bias), 0.0)
    ratelogA = node_arr(np.asarray(ratelog), 0.0)
    baserateA = node_arr(np.asarray(baserate), 0.0)
    capA = node_arr(np.asarray(cap), 1.0)

    xT4 = np.zeros((n_pad, ROWE), np.float32)
    xT4[:N, :B] = np.asarray(x, dtype=np.float32).T

    xTf = xT4[:, :B]
    xownA = np.zeros((n_cores, P, G, B), np.float32)
    node_ids = np.zeros((n_cores, P, G), np.int64)
    for c in range(n_cores):
        node_pos = ordq[c, 0]
        rank = node_pos * n_cores + c
        ok = rank < N
        nd = np.where(ok, order[np.minimum(rank, N - 1)], -1)
        jj = np.arange(nper)
        pcol = (jj % P, jj // P)
        node_ids[c][pcol] = nd
        xownA[c][pcol[0], pcol[1], :] = np.where(
            ok[:, None], xTf[np.maximum(nd, 0), :], 0.0)

    ins = []
    for c in range(n_cores):
        ins.append({
            "xT4": xT4,
            "gidx": gidx[c],
            "sidx": sidx[c],
            "theta": thetaA[c],
            "sgn": signA[c],
            "conf": confA[c],
            "delay": delayA[c],
            "maskf": maskA[c],
            "bias": biasA[c],
            "ratelog": ratelogA[c],
            "baserate": baserateA[c],
            "cap": capA[c],
            "xown": xownA[c].reshape(P, G * B),
        })
    plan = dict(B=B, N=N, G=G, nq=nq, D=D, S=S, F=F, Scol=Scol, Gact=Gact,
                n_pad=n_pad, gidx_cols=gidx_cols, sidx_cols=sidx_cols,
                chunks=chunks, sblocks=sblocks, sched=sched, gcol0=gcol0,
                scol0=scol0, node_ids=node_ids)
    return ins, plan


def _assemble(results, plan):
    B, N, G = plan["B"], plan["N"], plan["G"]
    out = np.empty((B, N), dtype=np.float32)
    for ci, res in enumerate(results):
        o = res["out"].reshape(P, G, B)
        nid = plan["node_ids"][ci]
        ok = nid >= 0
        out[:, nid[ok]] = o[ok].T
    return out


# ---------------------------------------------------------------------------
# Device kernel
# ---------------------------------------------------------------------------

def _raw_dma_gather(g, out_ap, in_ap, idxs_ap, num_idxs, num_idxs_reg,
                    elem_size, elem_step, queue_num):
    stride_bytes = elem_step * mybir.dt.size(in_ap.dtype)
    return g.add_instruction(
        mybir.InstDMAGatherAnt(
            name=g.bass.get_next_instruction_name(),
            ins=[*g.lower_ap_dma(in_ap, for_custom_bir_dma=True),
                 g.lower_ap(idxs_ap), g.lower_val_access(num_idxs_reg)],
            outs=[g.lower_ap(out_ap)],
            transpose=False, num_idxs=num_idxs, elem_size=elem_size,
            stride_bytes_256=stride_bytes // 256, gen_mode=0,
            single_packet=False, queue_num=queue_num,
            sbuf_tokens_per_rank=0, sbuf_free_dim_per_rank=0,
            sbuf_free_dim_pad_per_rank=0, sbuf_byte_offset=0))


def _equal_d_runs(D, g0, g1):
    runs = []
    a = g0
    while a < g1:
        b = a + 1
        while b < g1 and D[b] == D[a]:
            b += 1
        runs.append((a, b, int(D[a])))
        a = b
    return runs


def build(plan):
    B = plan["B"]
    G = plan["G"]
    nq = plan["nq"]
    D, S, F, Scol = plan["D"], plan["S"], plan["F"], plan["Scol"]
    n_pad = plan["n_pad"]
    chunks, sched = plan["chunks"], plan["sched"]
    sblocks = plan["sblocks"]
    gcol0, scol0 = plan["gcol0"], plan["scol0"]
    FT = int(Scol[-1])

    # distinct SWDGE call sizes -> one shared register each (written once;
    # per-call to_reg MOVEs rewrite a single GPR and Tile serializes every
    # call behind the previous one's completion sem to protect it)
    sizes = set()
    for kind, q, ci in sched:
        if kind == "g":
            sizes.add(chunks[q][ci][2] * P)
        else:
            g0, g1 = sblocks[q][ci]
            sizes.add((g1 - g0) * P)

    nc = bacc.Bacc("TRN2", target_bir_lowering=False, debug=False,
                   enable_asserts=False, num_swdge_queues=4,
                   dynamic_dma_scratch_size=RING_BYTES)

    xT4 = nc.dram_tensor("xT4", [n_pad, ROWE], F32, kind="ExternalInput")
    giD = nc.dram_tensor("gidx", [128, plan["gidx_cols"]], I16,
                         kind="ExternalInput")
    siD = nc.dram_tensor("sidx", [128, plan["sidx_cols"]], I16,
                         kind="ExternalInput")
    thD = nc.dram_tensor("theta", [P, FT], F32, kind="ExternalInput")
    sgD = nc.dram_tensor("sgn", [P, FT], F32, kind="ExternalInput")
    cfD = nc.dram_tensor("conf", [P, FT], F32, kind="ExternalInput")
    dlD = nc.dram_tensor("delay", [P, FT], F32, kind="ExternalInput")
    mkD = nc.dram_tensor("maskf", [P, FT], I8, kind="ExternalInput")
    biD = nc.dram_tensor("bias", [P, G], F32, kind="ExternalInput")
    rlD = nc.dram_tensor("ratelog", [P, G], F32, kind="ExternalInput")
    brD = nc.dram_tensor("baserate", [P, G], F32, kind="ExternalInput")
    cpD = nc.dram_tensor("cap", [P, G], F32, kind="ExternalInput")
    xoD = nc.dram_tensor("xown", [P, G * B], F32, kind="ExternalInput")
    outD = nc.dram_tensor("out", [P, G * B], F32, kind="ExternalOutput")
    # partial-agg merge buffers (zero-initialized by the runtime)
    pagg = [nc.dram_tensor(f"pagg{q}", [G * P, ROWE], F32,
                           kind="ExternalOutput") for q in range(1, nq)]

    Tanh = mybir.ActivationFunctionType.Tanh
    Exp = mybir.ActivationFunctionType.Exp

    with tile.TileContext(nc) as tc:
        with tc.tile_pool(name="persist", bufs=1) as pp:
            nreg = {v: nc.gpsimd.to_reg(v) for v in sorted(sizes)}

            gidx_t = pp.tile([128, plan["gidx_cols"]], I16, tag="gidx")
            nc.sync.dma_start(out=gidx_t[:], in_=giD[:, :])
            sidx_t = pp.tile([128, plan["sidx_cols"]], I16, tag="sidx")
            nc.sync.dma_start(out=sidx_t[:], in_=siD[:, :])

            bi = pp.tile([P, G], F32, tag="bi")
            rl = pp.tile([P, G], F32, tag="rl")
            br = pp.tile([P, G], F32, tag="br")
            cp = pp.tile([P, G], F32, tag="cp")
            xo = pp.tile([P, G * B], F32, tag="xo")
            nc.sync.dma_start(out=bi[:], in_=biD[:, :])
            nc.sync.dma_start(out=rl[:], in_=rlD[:, :])
            nc.sync.dma_start(out=br[:], in_=brD[:, :])
            nc.sync.dma_start(out=cp[:], in_=cpD[:, :])
            nc.sync.dma_start(out=xo[:], in_=xoD[:, :])

            # ---- edge weights, computed once ----
            w = pp.tile([P, FT], F32, tag="w")
            with tc.tile_pool(name="wprep", bufs=1) as wp:
                th = wp.tile([P, FT], F32, tag="th")
                sg = wp.tile([P, FT], F32, tag="sg")
                cf = wp.tile([P, FT], F32, tag="cf")
                dl = wp.tile([P, FT], F32, tag="dl")
                mk = wp.tile([P, FT], I8, tag="mk")
                nc.scalar.dma_start(out=th[:], in_=thD[:, :])
                nc.scalar.dma_start(out=sg[:], in_=sgD[:, :])
                nc.scalar.dma_start(out=cf[:], in_=cfD[:, :])
                nc.scalar.dma_start(out=dl[:], in_=dlD[:, :])
                nc.scalar.dma_start(out=mk[:], in_=mkD[:, :])
                t = wp.tile([P, FT], F32, tag="t")
                nc.scalar.activation(t[:], th[:], Tanh)
                nc.vector.tensor_mul(w[:], sg[:], cf[:])
                nc.vector.copy_predicated(w[:], mk[:], t[:])
                nc.vector.tensor_mul(w[:], w[:], dl[:])

            agg0 = pp.tile([P, G * B], F32, tag="agg0")
            aggq = {}
            for q in range(1, nq):
                aggq[q] = pp.tile([P, G * B], F32, tag=f"agg{q}",
                                  name=f"aggq{q}")

            qrr = [0]

            def next_q():
                r = qrr[0]
                qrr[0] = (r + 1) % 4
                return r

            with tc.tile_pool(name="msgs", bufs=12) as mp:
                for kind, q, ci in sched:
                    if kind == "g":
                        g0, g1, cols = chunks[q][ci]
                        slots = cols * P
                        m = mp.tile([P, CALL_COLS * B], F32, tag="m")
                        m3 = m[:, :cols * B].rearrange(
                            "p (s b) -> p s b", b=B)
                        base = q * SRC_CHUNK
                        in_ap = xT4[base:min(base + SRC_CHUNK, n_pad), :B]
                        gb = gcol0[(q, ci)]
                        _raw_dma_gather(
                            nc.gpsimd, m3, in_ap,
                            gidx_t[:, gb:gb + slots // 16],
                            slots, nreg[slots], B, ROWE, next_q())
                        # weight multiply (chunk slice of w, global cols)
                        c0, c1 = int(S[q, g0]), int(S[q, g1])
                        w0 = int(Scol[q])
                        wb = (w[:, w0 + c0:w0 + c1].unsqueeze(-1)
                              .to_broadcast([P, cols, B]))
                        nc.vector.tensor_mul(m3, m3, wb)
                        # segmented reduce into agg tile
                        aggt = agg0 if q == 0 else aggq[q]
                        for (ga, gb2, d) in _equal_d_runs(D[q], g0, g1):
                            src_ap = (m[:, (int(S[q, ga]) - c0) * B:
                                        (int(S[q, gb2]) - c0) * B]
                                      .rearrange("p (n d b) -> p n b d",
                                                 d=d, b=B))
                            dst_ap = aggt[:, ga * B:gb2 * B].rearrange(
                                "p (n b) -> p n b", b=B)
                            nc.vector.tensor_reduce(
                                dst_ap, src_ap, axis=mybir.AxisListType.X,
                                op=mybir.AluOpType.add)
                    else:
                        # merge scatter-add of aggq block into canonical order
                        g0, g1 = sblocks[q][ci]
                        nact = (g1 - g0) * P
                        a3 = aggq[q][:, g0 * B:g1 * B].rearrange(
                            "p (g b) -> p g b", b=B)
                        sb = scol0[(q, ci)]
                        nc.gpsimd.dma_scatter_add(
                            pagg[q - 1][:, :B], a3,
                            sidx_t[:, sb:sb + nact // 16],
                            nact, nreg[nact], B, elem_step=ROWE,
                            single_packet=False, queue_num=next_q())

            # ---- merge + ODE epilogue ----
            rdb = []
            for q in range(1, nq):
                rt = pp.tile([P, G * B], F32, tag=f"rdb{q}")
                nc.sync.dma_start(
                    out=rt[:].rearrange("p (g b) -> p g b", b=B),
                    in_=pagg[q - 1][:, :B].rearrange(
                        "(p g) b -> p g b", p=P))
                rdb.append(rt)
            for rt in rdb:
                nc.vector.tensor_add(agg0[:], agg0[:], rt[:])

            rate = pp.tile([P, G], F32, tag="rate")
            nc.scalar.activation(rate[:], rl[:], Exp)
            nc.vector.tensor_mul(rate[:], rate[:], br[:])

            a3 = agg0[:].rearrange("p (g b) -> p g b", b=B)
            bib = bi[:].unsqueeze(-1).to_broadcast([P, G, B])
            cpb = cp[:].unsqueeze(-1).to_broadcast([P, G, B])
            rateb = rate[:].unsqueeze(-1).to_broadcast([P, G, B])

            dr = pp.tile([P, G * B], F32, tag="dr")
            d3 = dr[:].rearrange("p (g b) -> p g b", b=B)
            nc.vector.tensor_add(d3, a3, bib)
            nc.scalar.activation(dr[:], dr[:], Tanh)
            nc.vector.tensor_mul(d3, d3, cpb)
            nc.vector.tensor_tensor(out=dr[:], in0=dr[:], in1=xo[:],
                                    op=mybir.AluOpType.subtract)
            nc.vector.tensor_mul(d3, d3, rateb)
            nc.vector.tensor_scalar_mul(dr[:], dr[:], float(DT))
            nc.vector.tensor_add(dr[:], dr[:], xo[:])
            nc.vector.tensor_scalar_max(dr[:], dr[:], 0.0)
            nc.vector.tensor_tensor(out=d3, in0=d3, in1=cpb,
                                    op=mybir.AluOpType.min)
            nc.sync.dma_start(out=outD[:, :], in_=dr[:])

    nc.compile()
    return nc


# ---------------------------------------------------------------------------
# Entry point
# ---------------------------------------------------------------------------

def kernel(x, theta_graph, node_bias, rate_log_scale, base_rate, capacity,
           sign_prior, conf_scale, delay_scale, src_index, dst_index,
           learn_mask):
    ins, plan = _prep(x, theta_graph, node_bias, rate_log_scale, base_rate,
                      capacity, sign_prior, conf_scale, delay_scale,
                      src_index, dst_index, learn_mask, NCORES)
    nc = build(plan)
    res = run_bass_kernel_spmd(nc, ins, core_ids=list(range(NCORES)))
    return _assemble(res.results, plan)
